# revision 10
# baseline (speedup 1.0000x reference)
"""GCN encoder (concat-edges GCNConv) as a distributed Bass/Tile kernel on 8 NeuronCores.

v2: fp16 datapath + SWDGE dma_gather (InstDMAGatherAnt) for the per-edge
feature gather instead of per-tile indirect DMAs.

Sharding: nodes/output sharded 8 ways (SH=12544/core); edges partitioned by
destination owner.  Per core, edges are bucketed by (dst 128-group g,
k = src % 4) and padded per (g, k) to 128-slot tiles so every tile is k-pure:
the gather fetches 4-row 256B elements (idx = src//4, int16) and the needed
row is a STATIC 32-column slice (k*32) of each gathered element.

Math:  out = dinv * (S @ (dinv * (x@W))) + dinv^2*(x@W) + b  with S the real
edge adjacency and dinv = rsqrt(deg+1); self-loops handled analytically.

Device does all FP math (h=x@W, degree histogram, rsqrt, prescale, AllGather,
gather+aggregate, final scale+bias).  Host only does layout: int64->int32,
edge bucketing/sorting, padding, int16 index packing, transposes.
"""
import sys

if "/opt/trn_rl_repo" not in sys.path:
    sys.path.insert(0, "/opt/trn_rl_repo")

import numpy as np

P = 128
LAT = 32
IN = 128
F = 32            # deg-pass block width
GTILES = 8        # tiles per dma_gather instruction (1024 idx, HW-validated max)
MC1 = 16          # deg-mask tiles per DVE build
MC2 = GTILES      # agg-mask tiles per DVE build (aligned with gather chunks)
G_BUFS = 20
NQ = 4


def _full_cfg():
    return dict(N=100_000, NC=8, SH=12_544)  # SH*NC = 100352, SH % 128 == 0


# ---------------------------------------------------------------- host layout
def prepare(x, edge_index, y_edge_index, W, b, cfg):
    N, NC, SH = cfg["N"], cfg["NC"], cfg["SH"]
    NPAD = NC * SH
    NG = SH // P          # 98 dst 128-groups per core
    B = SH // F           # 392 deg blocks per core

    SA = 8192                 # per-core rows in the first (early) AllGather
    SB = SH - SA              # 4352 rows in the second

    def pi(n):
        c = n // SH
        j = n % SH
        return np.where(j < SA, c * SA + j, NC * SA + c * SB + (j - SA))

    ei = np.concatenate([np.asarray(edge_index), np.asarray(y_edge_index)], axis=1)
    src_g = ei[0].astype(np.int64)
    dst_g = ei[1].astype(np.int64)
    owner = dst_g // SH

    per_core = []
    cnt1 = np.zeros((NC, B), np.int64)          # per 32-block (deg pass)
    cnt2 = np.zeros((NC, NG, 4), np.int64)      # per (128-group, k) (agg pass)
    for c in range(NC):
        sel = owner == c
        s = src_g[sel].astype(np.int64)
        d = (dst_g[sel] - c * SH).astype(np.int64)
        per_core.append((s, d))
        cnt1[c] = np.bincount(d // F, minlength=B)
        np.add.at(cnt2[c], (d // P, pi(s) >> 15), 1)

    # pass-1 (deg) stream: tiles per 32-block = max over cores
    T1b = np.ceil(cnt1.max(axis=0) / P).astype(np.int64)
    T1 = int(T1b.sum())
    st1 = np.concatenate([[0], np.cumsum(T1b)])
    # pass-2 (agg) stream: tiles per (g, k) = max over cores; k-major streams
    T2gk = np.ceil(cnt2.max(axis=0) / P).astype(np.int64)    # [NG, 4]
    Tks = [int(T2gk[:, k].sum()) for k in range(4)]
    # pad each range-stream to a whole number of gather chunks so every
    # dma_gather instruction reads from a single 32768-row table range
    Tkp = [((t + GTILES - 1) // GTILES) * GTILES for t in Tks]
    st2 = np.zeros((NG, 4), np.int64)   # global tile index of (g,k) run start
    off = 0
    kbase = []
    chunk_range = []
    for k in range(4):
        kbase.append(off)
        st2[:, k] = off + np.concatenate([[0], np.cumsum(T2gk[:, k])])[:-1]
        off += Tkp[k]
        chunk_range += [k] * (Tkp[k] // GTILES)
    T2 = off

    pad_idx = 0               # valid row in every range; masked out anyway

    x = np.asarray(x, np.float16)
    xpad = np.zeros((NPAD, IN), np.float16)
    xpad[:N] = x

    iota1 = np.tile(np.arange(F, dtype=np.float16)[None, :], (P, 1))  # [P, F]
    iota2 = np.repeat(np.arange(P, dtype=np.float16), MC2)
    iota2 = np.tile(iota2[None, :], (P, 1))                     # [P, P*MC2]
    ident32 = np.eye(F, dtype=np.float32)
    b32 = np.asarray(b, np.float32)
    b_rep = np.tile(b32[None, :], (P, 1))                       # [P, LAT] fp32
    W16 = np.asarray(W, np.float16)

    in_maps = []
    for c in range(NC):
        s, d = per_core[c]
        # ---- pass-1 stream (sorted by dst only)
        o1 = np.argsort(d, kind="stable")
        s1, d1 = s[o1], d[o1]
        blk = d1 // F
        run0 = np.concatenate([[0], np.cumsum(cnt1[c])[:-1]])
        slot = np.arange(len(d1)) - run0[blk]
        pos1 = (st1[blk] * P + slot).astype(np.int64)
        dr1 = np.full(T1 * P, 99.0, np.float16)
        dr1[pos1] = (d1 % F).astype(np.float16)
        dr1 = np.ascontiguousarray(dr1.reshape(T1, P).T)        # [P, T1]

        # ---- pass-2 stream (bucketed by (g, pi-range)), range-major
        ps_ = pi(s)
        k4 = (ps_ >> 15).astype(np.int64)
        g = d // P
        o2 = np.lexsort((d, g, k4))
        s2, d2, k2, g2 = ps_[o2], d[o2], k4[o2], g[o2]
        run0_2 = np.zeros((NG, 4), np.int64)
        csum = np.concatenate([[0], np.cumsum(cnt2[c].transpose(1, 0).reshape(-1))])
        # edges sorted by (k, g): run start of (k, g) in sorted order:
        run_start_kg = csum[:-1].reshape(4, NG)
        eidx = np.arange(len(d2))
        slot2 = eidx - run_start_kg[k2, g2]
        pos2 = (st2[g2, k2] * P + slot2).astype(np.int64)
        dr2 = np.full(T2 * P, 999.0, np.float16)
        dr2[pos2] = (d2 % P).astype(np.float16)
        dr2 = np.ascontiguousarray(dr2.reshape(T2, P).T)        # [P, T2]
        idxf = np.full(T2 * P, pad_idx, np.int32)
        idxf[pos2] = (s2 - (k2 << 15)).astype(np.int32)
        assert idxf.max() < (1 << 15) and idxf.min() >= 0
        # wrap per gather chunk of GTILES tiles (1024 idx), replicate stripes
        NIDX = GTILES * P
        nchunks_all = (T2 + GTILES - 1) // GTILES
        ncols = nchunks_all * (NIDX // 16)
        idx16 = np.zeros((P, ncols), np.int16)
        for ch in range(nchunks_all):
            lo = ch * NIDX
            hi = min(lo + NIDX, T2 * P)
            chunk = idxf[lo:hi]
            if len(chunk) < NIDX:
                chunk = np.concatenate(
                    [chunk, np.full(NIDX - len(chunk), pad_idx, np.int32)])
            w = chunk.astype(np.int16).reshape(-1, 16).T        # [16, NIDX//16]
            c0 = lo // 16
            for sst in range(8):
                idx16[sst * 16:(sst + 1) * 16, c0:c0 + NIDX // 16] = w

        xt = np.ascontiguousarray(xpad[c * SH:(c + 1) * SH].T)  # [IN, SH] fp16
        in_maps.append({
            "xT": xt,
            "dr1": dr1,
            "dr2": dr2,
            "idx16": idx16,
            "W": W16,
            "b_rep": b_rep,
            "iota1": iota1,
            "iota2": iota2,
            "ident32": ident32,
        })

    meta = dict(T1b=T1b.tolist(), T1=T1, T2gk=T2gk.tolist(), T2=T2,
                kbase=kbase, Tks=Tks, st1=st1.tolist(),
                st2=st2.tolist(), ncols=ncols, chunk_range=chunk_range)
    return in_maps, meta


# ---------------------------------------------------------------- device module
def build_module(cfg, meta):
    import concourse.bass as bass
    import concourse.bacc as bacc
    import concourse.tile as tile
    import concourse.mybir as mybir

    NC, SH = cfg["NC"], cfg["SH"]
    NPAD = NC * SH
    NG = SH // P
    B = SH // F
    T1, T2 = meta["T1"], meta["T2"]
    T1b = meta["T1b"]
    T2gk = meta["T2gk"]
    st1 = meta["st1"]
    st2 = meta["st2"]
    NIDX = GTILES * P

    nc = bacc.Bacc("TRN2", target_bir_lowering=False, debug=False,
                   enable_asserts=False, num_devices=NC,
                   num_swdge_queues=NQ, dynamic_dma_scratch_size=65536)

    dt = mybir.dt
    AF = mybir.ActivationFunctionType
    OP = mybir.AluOpType

    xT_d = nc.dram_tensor("xT", [IN, SH], dt.float16, kind="ExternalInput")
    dr1_d = nc.dram_tensor("dr1", [P, T1], dt.float16, kind="ExternalInput")
    dr2_d = nc.dram_tensor("dr2", [P, T2], dt.float16, kind="ExternalInput")
    idx_d = nc.dram_tensor("idx16", [P, meta["ncols"]], dt.int16,
                           kind="ExternalInput")
    W_d = nc.dram_tensor("W", [IN, LAT], dt.float16, kind="ExternalInput")
    brep_d = nc.dram_tensor("b_rep", [P, LAT], dt.float32, kind="ExternalInput")
    iota1_d = nc.dram_tensor("iota1", [P, F], dt.float16,
                             kind="ExternalInput")
    iota2_d = nc.dram_tensor("iota2", [P, P * MC2], dt.float16,
                             kind="ExternalInput")
    ident_d = nc.dram_tensor("ident32", [F, F], dt.float32,
                             kind="ExternalInput")
    out_d = nc.dram_tensor("out", [SH, LAT], dt.float32, kind="ExternalOutput")

    with tile.TileContext(nc) as tc:
        with tc.tile_pool(name="res", bufs=1) as res, \
             tc.tile_pool(name="dram", bufs=1, space="DRAM") as dram:
            dr1_t = res.tile([P, T1], dt.float16)
            dr2_t = res.tile([P, T2], dt.float16)
            idx_t = res.tile([P, meta["ncols"]], dt.int16)
            iota1_t = res.tile([P, F], dt.float16)
            iota2_t = res.tile([P, P * MC2], dt.float16)
            W_t = res.tile([IN, LAT], dt.float16)
            brep_t = res.tile([P, LAT], dt.float32)
            ones_t = res.tile([P, 1], dt.float16)
            h128 = res.tile([P, NG * LAT], dt.float16)
            acc128 = res.tile([P, NG * LAT], dt.float32)
            stage = res.tile([P, B], dt.float32)
            stage2 = res.tile([F, 4 * B], dt.float32)
            deg_sb = res.tile([F, B], dt.float32)
            dinv_sb = res.tile([F, B], dt.float32)
            dinv128 = res.tile([P, NG], dt.float32)
            warm = res.tile([P, 512], dt.float16)
            ident_t = res.tile([F, F], dt.float32)

            SA, SB = 8192, SH - 8192
            GA = SA // P                      # 64 groups in part a
            BA = SA // F                      # 256 deg blocks in part a
            h_shard_a = dram.tile([SA, LAT], dt.float16)
            h_shard_b = dram.tile([SB, LAT], dt.float16)
            h_full_a = dram.tile([NC * SA, LAT], dt.float16,
                                 addr_space="Shared")
            h_full_b = dram.tile([NC * SB, LAT], dt.float16,
                                 addr_space="Shared")
            # per-range 256B-stride row tables: range-r gathers only wait for
            # range-r's expand DMAs
            RR = 1 << 15
            rrows = [min(RR, NPAD - r * RR) for r in range(4)]
            h_pads = []
            for r in range(4):
                h_pad_r = dram.tile([rrows[r], P], dt.float16,
                                    name=f"h_pad_{r}")
                h_pads.append(h_pad_r)

            nc.sync.dma_start(dr1_t[:], dr1_d[:])
            nc.sync.dma_start(dr2_t[:], dr2_d[:])
            nc.sync.dma_start(idx_t[:], idx_d[:])
            nc.sync.dma_start(iota1_t[:], iota1_d[:])
            nc.sync.dma_start(iota2_t[:], iota2_d[:])
            nc.sync.dma_start(W_t[:], W_d[:])
            nc.sync.dma_start(brep_t[:], brep_d[:])
            nc.sync.dma_start(ident_t[:], ident_d[:])
            nc.vector.memset(ones_t[:], 1.0)
            nc.vector.memset(stage[:], 0.0)

            # ---------------- phase A: h = x @ W ---------------------------
            with tc.tile_pool(name="xt", bufs=1) as xtp, \
                 tc.tile_pool(name="psA", bufs=2, space="PSUM") as psA:
                nc.vector.memset(warm[:], 1.0)
                pw = psA.tile([P, 512], dt.float32, tag="h")
                for _ in range(10):
                    nc.tensor.matmul(out=pw[:], lhsT=warm[:, :P],
                                     rhs=warm[:], start=True, stop=True)
                nc.scalar.activation(warm[:, :1], pw[:, :1], AF.Copy)

                xT_t = xtp.tile([IN, SH], dt.float16)
                nc.sync.dma_start(xT_t[:], xT_d[:])
                for g in range(NG):
                    ph = psA.tile([P, LAT], dt.float32, tag="h")
                    nc.tensor.matmul(out=ph[:], lhsT=xT_t[:, g * P:(g + 1) * P],
                                     rhs=W_t[:], start=True, stop=True)
                    nc.scalar.activation(h128[:, g * LAT:(g + 1) * LAT],
                                         ph[:], AF.Copy)

            # ---------------- pass 1: degree histogram ---------------------
            with tc.tile_pool(name="m1", bufs=4) as mp1, \
                 tc.tile_pool(name="psD", bufs=8, space="PSUM") as psD:
                m1chunks = {}

                def mask1(j):
                    if j not in m1chunks:
                        cw = min(MC1, T1 - j * MC1)
                        mt = mp1.tile([P, MC1 * F], dt.float16, tag="m1")
                        nc.vector.tensor_tensor(
                            out=mt[:].rearrange("p (t d) -> p t d", d=F)
                                [:, :cw, :],
                            in0=dr1_t[:, j * MC1:j * MC1 + cw, None]
                                .to_broadcast([P, cw, F]),
                            in1=iota1_t[:, None, :].to_broadcast([P, cw, F]),
                            op=OP.is_equal)
                        m1chunks[j] = mt
                    return m1chunks[j]

                for bi in range(B):
                    t0, t1 = st1[bi], st1[bi + 1]
                    if t0 == t1:
                        continue
                    packs = []
                    t = t0
                    while t < t1:
                        lim = min(t1, (t // MC1 + 1) * MC1, t + 4)
                        packs.append((t, lim))
                        t = lim
                    packs.sort(key=lambda ab: ab[0] - ab[1])
                    maxrows = (packs[0][1] - packs[0][0]) * F
                    pd = psD.tile([P, 1], dt.float32, tag="deg")
                    for pi, (ta, tb_) in enumerate(packs):
                        mt = mask1(ta // MC1)
                        o = (ta % MC1) * F
                        nc.tensor.matmul(
                            out=pd[:(tb_ - ta) * F, :],
                            lhsT=mt[:, o:o + (tb_ - ta) * F],
                            rhs=ones_t[:],
                            start=(pi == 0), stop=(pi == len(packs) - 1),
                            skip_group_check=True)
                    nc.scalar.activation(stage[:maxrows, bi:bi + 1],
                                         pd[:maxrows, :], AF.Copy)

            # fold stage [128, B] -> deg_sb [32, B]
            for q in range(4):
                nc.sync.dma_start(stage2[:, q * B:(q + 1) * B],
                                  stage[q * F:(q + 1) * F, :])
            nc.vector.tensor_tensor(out=deg_sb[:], in0=stage2[:, 0:B],
                                    in1=stage2[:, B:2 * B], op=OP.add)
            nc.vector.tensor_tensor(out=stage2[:, 2 * B:3 * B],
                                    in0=stage2[:, 2 * B:3 * B],
                                    in1=stage2[:, 3 * B:4 * B], op=OP.add)
            nc.vector.tensor_tensor(out=deg_sb[:], in0=deg_sb[:],
                                    in1=stage2[:, 2 * B:3 * B], op=OP.add)

            # dinv = 1/sqrt(deg+1); h' = h*dinv; ship in two chunks. The
            # second AllGather is EMITTED mid-pass-A (gpsimd program order
            # otherwise gates the first gathers on its completion).
            for (b0, b1, g0, g1, shard, full, do_cc) in (
                    (0, BA, 0, GA, h_shard_a, h_full_a, True),
                    (BA, B, GA, NG, h_shard_b, h_full_b, False)):
                ngr = g1 - g0
                nc.scalar.activation(dinv_sb[:, b0:b1], deg_sb[:, b0:b1],
                                     AF.Sqrt, bias=1.0)
                nc.vector.reciprocal(dinv_sb[:, b0:b1], dinv_sb[:, b0:b1])
                for q in range(4):
                    nc.sync.dma_start(
                        dinv128[q * F:(q + 1) * F, g0:g1],
                        dinv_sb[:, b0:b1].rearrange(
                            "w (g four) -> w g four", four=4)[:, :, q])
                nc.vector.tensor_tensor(
                    out=h128[:, g0 * LAT:g1 * LAT]
                        .rearrange("p (g f) -> p g f", f=LAT),
                    in0=h128[:, g0 * LAT:g1 * LAT]
                        .rearrange("p (g f) -> p g f", f=LAT),
                    in1=dinv128[:, g0:g1, None].to_broadcast([P, ngr, LAT]),
                    op=OP.mult)
                nc.sync.dma_start(
                    shard[:].rearrange("(g p) f -> p g f", p=P),
                    h128[:, g0 * LAT:g1 * LAT]
                        .rearrange("p (g f) -> p g f", f=LAT))
                if do_cc:
                    nc.gpsimd.collective_compute(
                        "AllGather", OP.bypass,
                        replica_groups=[list(range(NC))],
                        ins=[shard[:]], outs=[full[:]])

            def emit_cc_b():
                nc.gpsimd.collective_compute(
                    "AllGather", OP.bypass,
                    replica_groups=[list(range(NC))],
                    ins=[h_shard_b[:]], outs=[h_full_b[:]])
                for r in (2, 3):
                    nrows = rrows[r]
                    half = nrows // 2
                    base = (r - 2) * RR
                    nc.sync.dma_start(h_pads[r][0:half, 0:LAT],
                                      h_full_b[base:base + half, :])
                    nc.scalar.dma_start(h_pads[r][half:nrows, 0:LAT],
                                        h_full_b[base + half:base + nrows, :])
            # expand ranges 0-1 now (gathers on them start right away);
            # ranges 2-3 are expanded by emit_cc_b() mid-pass-A
            for r in range(2):
                nrows = rrows[r]
                half = nrows // 2
                nc.sync.dma_start(
                    h_pads[r][0:half, 0:LAT],
                    h_full_a[r * RR:r * RR + half, :])
                nc.scalar.dma_start(
                    h_pads[r][half:nrows, 0:LAT],
                    h_full_a[r * RR + half:r * RR + nrows, :])

            # ---------------- pass 2: gather + aggregate -------------------
            T2gk_np = np.array(T2gk)
            st2_np = np.array(st2)
            with tc.tile_pool(name="gat", bufs=G_BUFS) as gp, \
                 tc.tile_pool(name="m2", bufs=12) as mp2, \
                 tc.tile_pool(name="psG", bufs=8, space="PSUM") as psG:
                gchunks = {}
                m2chunks = {}
                qctr = [0]

                chunk_range = meta["chunk_range"]

                def gather64(out_ap, in_ap, idxs_ap, queue_num):
                    gpe = nc.gpsimd
                    _in_ap = gpe.lower_ap_dma(in_ap, for_custom_bir_dma=True)
                    _idxs_ap = gpe.lower_ap(idxs_ap)
                    _out_ap = gpe.lower_ap(out_ap)
                    return gpe.add_instruction(
                        mybir.InstDMAGatherAnt(
                            name=nc.get_next_instruction_name(),
                            ins=[*_in_ap, _idxs_ap,
                                 gpe.lower_val_access(gpe.to_reg(NIDX))],
                            outs=[_out_ap],
                            transpose=False,
                            num_idxs=NIDX,
                            elem_size=LAT,
                            stride_bytes_256=1,
                            gen_mode=0,
                            single_packet=True,
                            queue_num=queue_num,
                            sbuf_tokens_per_rank=0,
                            sbuf_free_dim_per_rank=0,
                            sbuf_free_dim_pad_per_rank=0,
                            sbuf_byte_offset=0,
                        ))

                def gchunk(ci):
                    # gather chunk ci covers tiles [ci*GTILES, +GTILES)
                    if ci not in gchunks:
                        gt = gp.tile([P, GTILES * LAT], dt.float16, tag="g")
                        r = chunk_range[ci]
                        gather64(
                            out_ap=gt[:].rearrange("p (j e) -> p j e", e=LAT),
                            in_ap=h_pads[r][:, 0:LAT],
                            idxs_ap=idx_t[:, ci * (NIDX // 16):
                                          (ci + 1) * (NIDX // 16)],
                            queue_num=qctr[0] % NQ,
                        )
                        qctr[0] += 1
                        gchunks[ci] = gt
                    return gchunks[ci]

                def mask2(j):
                    if j not in m2chunks:
                        cw = min(MC2, T2 - j * MC2)
                        mt = mp2.tile([P, P * MC2], dt.float16, tag="m2")
                        nc.vector.tensor_tensor(
                            out=mt[:].rearrange("p (d t) -> p d t", t=MC2)
                                [:, :, :cw],
                            in0=dr2_t[:, None, j * MC2:j * MC2 + cw]
                                .to_broadcast([P, P, cw]),
                            in1=iota2_t[:].rearrange("p (d t) -> p d t",
                                                     t=MC2)[:, :, :cw],
                            op=OP.is_equal)
                        m2chunks[j] = mt
                    return m2chunks[j]

                cc_b_done = [False]
                for kpair, pass_b in (((0, 1), False), ((2, 3), True)):
                    for g in range(NG):
                        if not pass_b and g == 20 and not cc_b_done[0]:
                            emit_cc_b()
                            cc_b_done[0] = True
                        tlist = []
                        for k in kpair:
                            t0 = st2_np[g, k]
                            tlist += [(int(t), k) for t in
                                      range(t0, t0 + int(T2gk_np[g, k]))]
                        if not tlist:
                            continue
                        pa = psG.tile([P, LAT], dt.float32, tag="agg")
                        for ti, (t, k) in enumerate(tlist):
                            gt = gchunk(t // GTILES)
                            mt = mask2(t // MC2)
                            nc.tensor.matmul(
                                out=pa[:],
                                lhsT=mt[:].rearrange("p (d t) -> p d t",
                                                     t=MC2)[:, :, t % MC2],
                                rhs=gt[:, (t % GTILES) * LAT:
                                       (t % GTILES + 1) * LAT],
                                start=(ti == 0),
                                stop=(ti == len(tlist) - 1))
                        if not pass_b:
                            nc.scalar.activation(
                                acc128[:, g * LAT:(g + 1) * LAT],
                                pa[:], AF.Copy)
                        else:
                            nc.vector.tensor_tensor(
                                out=acc128[:, g * LAT:(g + 1) * LAT],
                                in0=acc128[:, g * LAT:(g + 1) * LAT],
                                in1=pa[:], op=OP.add)

            # ---------------- finalize: out = dinv*(acc + h') + b ----------
            nc.vector.tensor_tensor(
                out=acc128[:].rearrange("p (g f) -> p g f", f=LAT),
                in0=acc128[:].rearrange("p (g f) -> p g f", f=LAT),
                in1=h128[:].rearrange("p (g f) -> p g f", f=LAT),
                op=OP.add)
            nc.vector.tensor_tensor(
                out=acc128[:].rearrange("p (g f) -> p g f", f=LAT),
                in0=acc128[:].rearrange("p (g f) -> p g f", f=LAT),
                in1=dinv128[:, :, None].to_broadcast([P, NG, LAT]),
                op=OP.mult)
            nc.vector.tensor_tensor(
                out=acc128[:].rearrange("p (g f) -> p g f", f=LAT),
                in0=acc128[:].rearrange("p (g f) -> p g f", f=LAT),
                in1=brep_t[:, None, :].to_broadcast([P, NG, LAT]),
                op=OP.add)
            nc.sync.dma_start(
                out_d.rearrange("(g p) f -> p g f", p=P),
                acc128[:].rearrange("p (g f) -> p g f", f=LAT))

    nc.compile()
    return nc


# ---------------------------------------------------------------- entry point
LAST_EXEC_NS = None


def kernel(x, edge_index, y_edge_index, W, b):
    import os
    global LAST_EXEC_NS
    from concourse import bass_utils

    cfg = _full_cfg()
    in_maps, meta = prepare(x, edge_index, y_edge_index, W, b, cfg)
    nc = build_module(cfg, meta)
    trace = os.environ.get("KERNEL_TRACE", "0") == "1"
    res = bass_utils.run_bass_kernel_spmd(nc, in_maps,
                                          core_ids=list(range(cfg["NC"])),
                                          trace=trace)
    if trace:
        LAST_EXEC_NS = res.exec_time_ns
        print("exec_time_ns:", res.exec_time_ns, flush=True)
    outs = [res.results[c]["out"] for c in range(cfg["NC"])]
    return np.concatenate(outs, axis=0)[:cfg["N"]].astype(np.float32)
